# revision 1
# baseline (speedup 1.0000x reference)
"""HalfKP input layer (dual GEMV + bias + relu) on 8 Trainium2 NeuronCores.

out[512] = concat(relu(W_my @ x[:41024] + b_my), relu(W_opp @ x[41024:] + b_opp))

Sharding: 512 output rows split 64 rows/core (output-feature parallel; cores
0-3 handle W_my, 4-7 handle W_opp).  Per core the [64, 41024] shard is
host-repacked into [128, 16*1282]: partition p = rr*32 + b holds row
(t*4 + rr)'s k-block b (kb=1282) at free offset t*1282.  The device streams W
in 1.31 MB DMAs (10.3 KB contiguous runs per partition), runs 16 fused
multiply+reduce custom-DVE ops (TENSOR_TENSOR_REDUCE against a [128, 1282]
x-block tile, bias seeded via s0), contracts the 32 k-block partials per row
with one tiny PE matmul, applies relu on DVE, and writes a [4, 16] result per
core.  Memory-roofline bound: ~10.5 MB HBM reads per core (~400 GB/s/core
measured stream rate).
"""

import numpy as np

K = 41024          # features per side
B = 32             # k-blocks per row
KB = K // B        # 1282 elements per k-block
R = 128 // B       # 4 rows processed per DVE op
T = 64 // R        # 16 DVE ops (row groups) per core
CHUNK = 2          # TTR ops per W DMA (10.3KB contiguous runs per partition)
XCOLS = KB + R + T  # xq | mask[4] | seed[16]
N_CORES = 8
ROWS_PER_CORE = 64

_compiled = None


def _build_nc():
    import concourse.bacc as bacc
    import concourse.mybir as mybir
    import concourse.tile as tile
    from concourse.dve_ops import TENSOR_TENSOR_REDUCE

    F32 = mybir.dt.float32

    nc = bacc.Bacc("TRN2", target_bir_lowering=False, debug=False)

    wt_d = nc.dram_tensor("wt", [128, T * KB], F32, kind="ExternalInput")
    xqp_d = nc.dram_tensor("xqp", [128, XCOLS], F32, kind="ExternalInput")
    out_d = nc.dram_tensor("out", [R, T], F32, kind="ExternalOutput")

    def ttr(w_ap, xq_ap, seed_ap, acc_ap, prod_ap):
        # out = in0*in1*s1; accum = s0 + sum(out)  (custom-DVE ucode op)
        nc.vector._custom_dve(
            TENSOR_TENSOR_REDUCE,
            out=prod_ap,
            in0=w_ap,
            in1=xq_ap,
            s0=seed_ap,
            s1=1.0,
            accum_out=acc_ap,
        )

    n_chunks = T // CHUNK
    with tile.TileContext(nc) as tc:
        with (
            tc.tile_pool(name="const", bufs=1) as constp,
            tc.tile_pool(name="w", bufs=n_chunks + 1) as wp,
            tc.tile_pool(name="scratch", bufs=1) as sp,
            tc.tile_pool(name="ps", bufs=1, space="PSUM") as psp,
        ):
            # xqp rides the scalar (ACT) HWDGE ring so the W stream on the
            # sync ring starts immediately and both make progress in parallel
            xqp = constp.tile([128, XCOLS], F32, tag="xqp")
            nc.scalar.dma_start(xqp[:], xqp_d[:])
            xq = xqp[:, 0:KB]
            mask = xqp[:, KB : KB + R]
            seed = xqp[:, KB + R : KB + R + T]

            acc = constp.tile([128, T], F32, tag="acc")
            prod = sp.tile([128, KB], F32, tag="prod")

            for c in range(n_chunks):
                w_sb = wp.tile([128, CHUNK * KB], F32, tag="w")
                nc.sync.dma_start(
                    w_sb[:], wt_d[:, c * CHUNK * KB : (c + 1) * CHUNK * KB]
                )
                for j in range(CHUNK):
                    t = c * CHUNK + j
                    ttr(
                        w_sb[:, j * KB : (j + 1) * KB],
                        xq,
                        seed[:, t : t + 1],
                        acc[:, t : t + 1],
                        prod[:],
                    )

            ps = psp.tile([R, T], F32, tag="ps")
            nc.tensor.matmul(ps[:], lhsT=mask, rhs=acc[:], start=True, stop=True)
            out_sb = sp.tile([R, T], F32, tag="out")
            nc.vector.tensor_scalar_max(out_sb[:], ps[:], 0.0)
            nc.sync.dma_start(out_d[:], out_sb[:])

    nc.compile()
    return nc


def _get_nc():
    global _compiled
    if _compiled is None:
        _compiled = _build_nc()
    return _compiled


def make_in_maps(input, W_my, b_my, W_opp, b_opp):
    """Host-side sharding: per-core input dicts."""
    x = np.ascontiguousarray(input, dtype=np.float32)
    Wcat = np.concatenate(
        [np.asarray(W_my, np.float32), np.asarray(W_opp, np.float32)], axis=0
    )
    bcat = np.concatenate(
        [np.asarray(b_my, np.float32), np.asarray(b_opp, np.float32)]
    )

    mask = (np.arange(128)[:, None] // B == np.arange(R)[None, :]).astype(np.float32)

    in_maps = []
    for c in range(N_CORES):
        Wsh = Wcat[c * ROWS_PER_CORE : (c + 1) * ROWS_PER_CORE]  # [64, K]
        xs = x[:K] if c < 4 else x[K:]
        # wt[p = rr*B + b, t*KB + j] = Wsh[t*R + rr, b*KB + j]
        wt = np.ascontiguousarray(
            Wsh.reshape(T, R, B, KB).transpose(1, 2, 0, 3).reshape(128, T * KB)
        )
        bsh = bcat[c * ROWS_PER_CORE : (c + 1) * ROWS_PER_CORE]
        seed = np.zeros((128, T), np.float32)
        # partition rr*B (b == 0) seeds the bias for row t*R + rr
        seed[np.arange(R) * B, :] = bsh.reshape(T, R).T
        xqp = np.empty((128, XCOLS), np.float32)
        xqp[:, 0:KB] = np.tile(xs.reshape(B, KB), (R, 1))
        xqp[:, KB : KB + R] = mask
        xqp[:, KB + R :] = seed
        in_maps.append({"wt": wt, "xqp": xqp})
    return in_maps


def gather_output(results):
    """results: list of per-core dicts with 'out' [R, T] -> full [512]."""
    outs = []
    for c in range(N_CORES):
        o = np.asarray(results[c]["out"], np.float32)  # [R, T]
        outs.append(o.T.ravel())  # row r = t*R + rr
    return np.concatenate(outs)


def run_on_hw(in_maps, trace=False, **kwargs):
    from concourse.bass_utils import run_bass_kernel_spmd

    nc = _get_nc()
    return run_bass_kernel_spmd(
        nc, in_maps, core_ids=list(range(N_CORES)), trace=trace, **kwargs
    )


def kernel(input, W_my, b_my, W_opp, b_opp):
    in_maps = make_in_maps(input, W_my, b_my, W_opp, b_opp)
    res = run_on_hw(in_maps)
    return gather_output(res.results)



# revision 7
# speedup vs baseline: 1.1954x; 1.1954x over previous
"""HalfKP input layer (dual GEMV + bias + relu) on 8 Trainium2 NeuronCores.

out[512] = concat(relu(W_my @ x[:41024] + b_my), relu(W_opp @ x[41024:] + b_opp))

Sharding: 512 output rows split 64 rows/core (cores 0-3: W_my, 4-7: W_opp).

Per-core compute is an x-stationary PE GEMV: K is split into 321 blocks of
128 (tail zero-padded).  For each chunk of I=8 blocks the PE loads 8 x-blocks
as the stationary operand [128, 8] and streams the chunk's W as the moving
operand [128, 512] (col j*64+r = row r's block j), accumulating psum[i, n] =
<x-block i, W-col n> over 40 chunks.  Only the diagonal i==j entries are
needed; 8 DVE copies extract them into acc[10, 64] (plus the tail partial and
the bias row), and a ones-vector f32 matmul contracts the 10 partials into
[1, 64], followed by relu and one contiguous 256 B store.
"""

import numpy as np
import ml_dtypes

K = 41024            # features per side
NB = 321             # 128-wide k-blocks per row (block 320 is the 64-wide tail)
I = 8                # x-blocks per stationary load
NCH = 40             # full chunks (8 blocks each)
MOV = I * 64         # moving columns per chunk
WCOLS = NCH * MOV + 64   # 20544
DGRP = 4             # chunks per W DMA
N_CORES = 8
ROWS = 64            # output rows per core

_compiled = None


def _build_nc():
    import concourse.bacc as bacc
    import concourse.mybir as mybir
    import concourse.tile as tile

    F32 = mybir.dt.float32
    BF16 = mybir.dt.bfloat16

    nc = bacc.Bacc("TRN2", target_bir_lowering=False, debug=False)

    wt_d = nc.dram_tensor("wt", [128, WCOLS], BF16, kind="ExternalInput")
    xs_d = nc.dram_tensor("xs", [128, NB], BF16, kind="ExternalInput")
    b_d = nc.dram_tensor("b", [1, ROWS], F32, kind="ExternalInput")
    sel_d = nc.dram_tensor("sel", [8, 8], F32, kind="ExternalInput")
    out_d = nc.dram_tensor("out", [1, ROWS], F32, kind="ExternalOutput")

    n_dma = NCH // DGRP
    with tile.TileContext(nc) as tc:
        with (
            tc.tile_pool(name="const", bufs=1) as constp,
            tc.tile_pool(name="w", bufs=3) as wp,
            tc.tile_pool(name="scratch", bufs=1) as sp,
            tc.tile_pool(name="ps", bufs=1, space="PSUM") as psp,
        ):
            # x + bias ride the scalar (ACT) HWDGE ring so the W stream on
            # the sync ring starts immediately and both progress in parallel
            xs = constp.tile([128, NB], BF16, tag="xs")
            nc.scalar.dma_start(xs[:], xs_d[:])
            # sel[:, j] = e_j — selects psum partition j in the extraction
            # matmuls (engines can't read PSUM at partition base j directly)
            sel = constp.tile([8, 8], F32, tag="sel")
            nc.scalar.dma_start(sel[:], sel_d[:])
            bias = constp.tile([1, ROWS], F32, tag="bias")
            nc.scalar.dma_start(bias[:], b_d[:])

            ps = psp.tile([I, MOV], F32, tag="ps")
            ps_t = psp.tile([1, 64], F32, tag="ps_t")
            ps_r = psp.tile([1, ROWS], F32, tag="ps_r")

            for d in range(n_dma):
                w_sb = wp.tile([128, DGRP * MOV], BF16, tag="w")
                nc.sync.dma_start(
                    w_sb[:], wt_d[:, d * DGRP * MOV : (d + 1) * DGRP * MOV]
                )
                for g in range(DGRP):
                    c = d * DGRP + g
                    nc.tensor.matmul(
                        ps[:],
                        lhsT=xs[:, c * I : (c + 1) * I],
                        rhs=w_sb[:, g * MOV : (g + 1) * MOV],
                        start=(c == 0),
                        stop=(c == NCH - 1),
                    )

            # tail block (k = 40960..41023, zero-padded to 128)
            w_tl = sp.tile([128, 64], BF16, tag="wtail")
            nc.sync.dma_start(w_tl[:], wt_d[:, NCH * MOV :])
            nc.tensor.matmul(
                ps_t[:], lhsT=xs[:, NB - 1 : NB], rhs=w_tl[:], start=True, stop=True
            )

            # tail partial + bias combine off the critical path
            tb = sp.tile([1, ROWS], F32, tag="tb")
            nc.vector.tensor_tensor(
                tb[:], ps_t[:], bias[:], op=mybir.AluOpType.add
            )

            # psum -> SBUF (partition-aligned), then extract the diagonal
            # blocks via 8 selector matmuls accumulating into ps_r
            sb8 = sp.tile([I, MOV], F32, tag="sb8")
            nc.vector.tensor_copy(sb8[:], ps[:])
            for j in range(I):
                nc.tensor.matmul(
                    ps_r[:],
                    lhsT=sel[:, j : j + 1],
                    rhs=sb8[:, j * 64 : (j + 1) * 64],
                    start=(j == 0),
                    stop=(j == I - 1),
                )

            out_sb = sp.tile([1, ROWS], F32, tag="out")
            v = sp.tile([1, ROWS], F32, tag="v")
            nc.vector.tensor_tensor(v[:], ps_r[:], tb[:], op=mybir.AluOpType.add)
            nc.vector.tensor_scalar_max(out_sb[:], v[:], 0.0)
            nc.sync.dma_start(out_d[:], out_sb[:])

    nc.compile()
    return nc


def _get_nc():
    global _compiled
    if _compiled is None:
        _compiled = _build_nc()
    return _compiled


def make_in_maps(input, W_my, b_my, W_opp, b_opp):
    """Host-side sharding: per-core input dicts."""
    x = np.ascontiguousarray(input, dtype=np.float32)
    Wcat = np.concatenate(
        [np.asarray(W_my, np.float32), np.asarray(W_opp, np.float32)], axis=0
    )
    bcat = np.concatenate(
        [np.asarray(b_my, np.float32), np.asarray(b_opp, np.float32)]
    )

    in_maps = []
    for c in range(N_CORES):
        Wsh = Wcat[c * ROWS : (c + 1) * ROWS]          # [64, K] f32
        xs_side = x[:K] if c < 4 else x[K:]

        # moving W: wt[p, c*512 + j*64 + r] = Wsh[r, (c*8+j)*128 + p]
        wm = (
            Wsh[:, : NCH * I * 128]
            .reshape(ROWS, NCH, I, 128)
            .transpose(3, 1, 2, 0)
            .reshape(128, NCH * MOV)
        )
        wt = np.zeros((128, WCOLS), np.float32)
        wt[:, : NCH * MOV] = wm
        wt[:64, NCH * MOV :] = Wsh[:, NCH * I * 128 :].T   # tail, zero-padded
        wt = wt.astype(ml_dtypes.bfloat16)

        xp = np.zeros(NB * 128, np.float32)
        xp[:K] = xs_side
        xs = np.ascontiguousarray(xp.reshape(NB, 128).T).astype(ml_dtypes.bfloat16)

        b = np.ascontiguousarray(bcat[c * ROWS : (c + 1) * ROWS].reshape(1, ROWS))
        in_maps.append({"wt": wt, "xs": xs, "b": b, "sel": np.eye(8, dtype=np.float32)})
    return in_maps


def gather_output(results):
    """results: list of per-core dicts with 'out' [1, 64] -> full [512]."""
    return np.concatenate(
        [np.asarray(results[c]["out"], np.float32).ravel() for c in range(N_CORES)]
    )


def run_on_hw(in_maps, trace=False, **kwargs):
    from concourse.bass_utils import run_bass_kernel_spmd

    nc = _get_nc()
    return run_bass_kernel_spmd(
        nc, in_maps, core_ids=list(range(N_CORES)), trace=trace, **kwargs
    )


def kernel(input, W_my, b_my, W_opp, b_opp):
    in_maps = make_in_maps(input, W_my, b_my, W_opp, b_opp)
    res = run_on_hw(in_maps)
    return gather_output(res.results)
